# revision 6
# baseline (speedup 1.0000x reference)
"""Trainium2 Bass kernel for GroupNorm -> self-attention -> proj + residual.

Reference computation (per image, b=32 total, data-parallel over 8 cores):
    xn    = GroupNorm(x, 8 groups, affine)              [c=256, n=1024]
    qkv   = qkv_w @ xn + qkv_b                          (1x1 conv)
    st    = k^T q / sqrt(c)   (scores transposed: [nk, nq])
    est   = exp(st)           (softmax without max-subtraction; randn data)
    den   = colsum(est)       (ones-matmul, replicated across partitions)
    outT  = (v^T est) / den   [c, nq]
    fin   = out_w @ outT + (out_w @ v_b + out_b) + xn

Layout choices:
  - x/xn/q/k/outT/final live as [c-half on partitions, n free] (2 tiles).
  - v is produced directly transposed ([n on partitions, c free]) by using
    xn as the matmul lhsT, so no on-chip transposes are needed anywhere.
  - softmax denominator: ones[128,128] lhsT matmul accumulated over the 8
    est K-tiles gives the column sum replicated on every partition, which a
    plain tensor_tensor multiply can then consume.
  - 1/den is folded into the outT PSUM->SBUF copyback; the v-bias and
    out-proj bias fold into a host-precomputed per-channel vector fb,
    which is folded into the GroupNorm-residual tile (xn + fb).
Matmul inputs are bf16 (fp32 accumulation in PSUM); GroupNorm statistics,
softmax sums and the residual path stay fp32.
"""

import numpy as np
import ml_dtypes
from contextlib import ExitStack

import concourse.bass as bass
import concourse.tile as tile
import concourse.mybir as mybir
from concourse import bacc
from concourse.bass import ts
from concourse.bass_utils import run_bass_kernel_spmd

P = 128
N_CORES = 8
B, C, H, W = 32, 256, 32, 32
N = H * W                      # 1024 pixels
IMGS = B // N_CORES            # 4 images per core
NH = C // P                    # 2 channel halves
NT = N // P                    # 8 pixel tiles
GROUPS = 8
EPS = 1e-5
F32 = mybir.dt.float32
BF16 = mybir.dt.bfloat16
AF = mybir.ActivationFunctionType
OP = mybir.AluOpType
CHUNK = 512                    # matmul moving free dim (one PSUM bank)
NCH = N // CHUNK               # 2 chunks


def _emit(ctx: ExitStack, tc: tile.TileContext, t: dict):
    nc = tc.nc

    singles = ctx.enter_context(tc.tile_pool(name="singles", bufs=1))
    p_x = ctx.enter_context(tc.tile_pool(name="p_x", bufs=2))
    p_stats = ctx.enter_context(tc.tile_pool(name="p_stats", bufs=4))
    p_xnb = ctx.enter_context(tc.tile_pool(name="p_xnb", bufs=2))
    p_xnfb = ctx.enter_context(tc.tile_pool(name="p_xnfb", bufs=2))
    p_qk = ctx.enter_context(tc.tile_pool(name="p_qk", bufs=2))
    p_vt = ctx.enter_context(tc.tile_pool(name="p_vt", bufs=2))
    p_est = ctx.enter_context(tc.tile_pool(name="p_est", bufs=2))
    p_recip = ctx.enter_context(tc.tile_pool(name="p_recip", bufs=2))
    p_outt = ctx.enter_context(tc.tile_pool(name="p_outt", bufs=2))
    p_fin = ctx.enter_context(tc.tile_pool(name="p_fin", bufs=4))
    ps_big = ctx.enter_context(tc.tile_pool(name="ps_big", bufs=3, space="PSUM"))
    ps_sm = ctx.enter_context(tc.tile_pool(name="ps_sm", bufs=2, space="PSUM"))

    # ---- load constants / weights into SBUF once ----
    s_wqkT = singles.tile([P, NH, 512], BF16)
    nc.sync.dma_start(s_wqkT[:], t["wqkT"].rearrange("h p o -> p h o"))
    s_wvT = singles.tile([P, NH, C], BF16)
    nc.sync.dma_start(s_wvT[:], t["wvT"].rearrange("h p o -> p h o"))
    s_woT = singles.tile([P, NH, C], BF16)
    nc.sync.dma_start(s_woT[:], t["woT"].rearrange("h p o -> p h o"))
    s_bqk = singles.tile([P, 4], F32)
    nc.sync.dma_start(s_bqk[:], t["bqk"].rearrange("j p -> p j"))
    s_gnw = singles.tile([P, NH], F32)
    nc.sync.dma_start(s_gnw[:], t["gnw"].rearrange("h p -> p h"))
    s_gnbfb = singles.tile([P, NH, 2], F32)  # col0 = gn_b, col1 = gn_b + fb
    nc.sync.dma_start(s_gnbfb[:], t["gnbfb"].rearrange("h p k -> p h k"))
    s_ind = singles.tile([P, NH, GROUPS], F32)
    nc.sync.dma_start(s_ind[:], t["ind"].rearrange("h p g -> p h g"))
    s_indT = singles.tile([GROUPS, NH, P], F32)
    nc.sync.dma_start(s_indT[:], t["indT"])
    s_ones = singles.tile([P, P], BF16)
    nc.vector.memset(s_ones[:], 1.0)
    s_eps = singles.tile([GROUPS, 1], F32)
    nc.vector.memset(s_eps[:], EPS)

    x_ap = t["x"]       # [IMGS, NH, P, N]
    out_ap = t["out"]   # [IMGS, NH, P, N]

    for img in range(IMGS):
        # ---------------- GroupNorm ----------------
        x_t = p_x.tile([P, NH, N], F32, tag="x")
        nc.sync.dma_start(x_t[:], x_ap[img].rearrange("h p n -> p h n"))

        # per-channel mean/var via bn_stats (free dim limited to 512)
        mm = p_stats.tile([P, NH, 2], F32, tag="mm")  # col0 mean, col1 E[x^2]
        for h in range(NH):
            st6 = p_stats.tile([P, 2, 6], F32, tag="st6")
            xv = x_t[:, h].rearrange("p (s f) -> p s f", f=512)
            for s in range(2):
                nc.vector.bn_stats(out=st6[:, s, :], in_=xv[:, s, :])
            mv = p_stats.tile([P, 2], F32, tag="mv")
            nc.vector.bn_aggr(out=mv[:], in_=st6[:])
            nc.vector.tensor_copy(mm[:, h, 0:1], mv[:, 0:1])
            # E[x^2] = var + mean^2
            nc.vector.tensor_tensor(mm[:, h, 1:2], mv[:, 0:1], mv[:, 0:1], OP.mult)
            nc.vector.tensor_tensor(mm[:, h, 1:2], mm[:, h, 1:2], mv[:, 1:2], OP.add)

        # group stats: [8, 2] = sum_h ind[h].T @ mm[h]  (values 1/32 -> mean)
        psg = ps_sm.tile([GROUPS, 2], F32, tag="sm")
        for h in range(NH):
            nc.tensor.matmul(psg[:], s_ind[:, h, :], mm[:, h, :],
                             start=(h == 0), stop=(h == NH - 1))
        grp = p_stats.tile([GROUPS, 2], F32, tag="grp")  # col0 mu, col1 rstd
        nc.vector.tensor_copy(grp[:, 0:1], psg[:, 0:1])
        nc.vector.tensor_copy(grp[:, 1:2], psg[:, 1:2])
        musq = p_stats.tile([GROUPS, 1], F32, tag="musq")
        nc.vector.tensor_tensor(musq[:], grp[:, 0:1], grp[:, 0:1], OP.mult)
        nc.vector.tensor_tensor(grp[:, 1:2], grp[:, 1:2], musq[:], OP.subtract)
        nc.scalar.activation(out=grp[:, 1:2], in_=grp[:, 1:2], func=AF.Sqrt,
                             bias=s_eps[:], scale=1.0)
        nc.vector.reciprocal(grp[:, 1:2], grp[:, 1:2])

        # broadcast group (mu, rstd) back to channels and build affine a,b
        ab = p_stats.tile([P, NH, 3], F32, tag="ab")  # a, b, b+fb
        for h in range(NH):
            psb = ps_sm.tile([P, 2], F32, tag="sm")
            nc.tensor.matmul(psb[:], s_indT[:, h, :], grp[:], start=True, stop=True)
            a = ab[:, h, 0:1]
            nc.vector.tensor_tensor(a, psb[:, 1:2], s_gnw[:, h:h + 1], OP.mult)
            mua = ab[:, h, 1:2]
            nc.vector.tensor_tensor(mua, psb[:, 0:1], a, OP.mult)
            # b = gn_b - mu*a ; b_fb = (gn_b + fb) - mu*a
            nc.vector.tensor_tensor(ab[:, h, 2:3], s_gnbfb[:, h, 1:2], mua, OP.subtract)
            nc.vector.tensor_tensor(mua, s_gnbfb[:, h, 0:1], mua, OP.subtract)

        # apply: xnb = bf16(x*a + b) on ACT; xnfb = f32(x*a + (b+fb)) on DVE
        xnb = p_xnb.tile([P, NH, N], BF16, tag="xnb")
        xnfb = p_xnfb.tile([P, NH, N], F32, tag="xnfb")
        for h in range(NH):
            nc.scalar.activation(out=xnb[:, h], in_=x_t[:, h], func=AF.Identity,
                                 bias=ab[:, h, 1:2], scale=ab[:, h, 0:1])
            nc.vector.tensor_scalar(out=xnfb[:, h], in0=x_t[:, h],
                                    scalar1=ab[:, h, 0:1], scalar2=ab[:, h, 2:3],
                                    op0=OP.mult, op1=OP.add)

        # ---------------- QKV ----------------
        # q,k in [c, n] layout: psum[j] = sum_h wqkT[:,h,128j:].T @ xnb[:,h,:]
        qk = p_qk.tile([P, 4, N], BF16, tag="qk")  # j=0,1 -> q ; j=2,3 -> k
        for j in range(4):
            ps = ps_big.tile([P, N], F32, tag="big")
            for ch in range(NCH):
                for h in range(NH):
                    nc.tensor.matmul(ps[:, ts(ch, CHUNK)],
                                     s_wqkT[:, h, ts(j, P)],
                                     xnb[:, h, ts(ch, CHUNK)],
                                     start=(h == 0), stop=(h == NH - 1))
            nc.any.tensor_scalar(out=qk[:, j], in0=ps[:],
                                 scalar1=s_bqk[:, j:j + 1], scalar2=None,
                                 op0=OP.add)

        # vT in [n, c] layout: psum[t] = sum_h xnb[:,h,128t:].T @ wvT[:,h,:]
        vt = p_vt.tile([P, NT, C], BF16, tag="vt")
        for tt in range(NT):
            ps = ps_sm.tile([P, C], F32, tag="sm")
            for h in range(NH):
                nc.tensor.matmul(ps[:], xnb[:, h, ts(tt, P)], s_wvT[:, h, :],
                                 start=(h == 0), stop=(h == NH - 1))
            nc.any.tensor_copy(out=vt[:, tt], in_=ps[:])

        # ---------------- scores^T + exp ----------------
        est = p_est.tile([P, NT, N], BF16, tag="est")
        for tt in range(NT):
            ps = ps_big.tile([P, N], F32, tag="big")
            for ch in range(NCH):
                for h in range(NH):
                    nc.tensor.matmul(ps[:, ts(ch, CHUNK)],
                                     qk[:, 2 + h, ts(tt, P)],
                                     qk[:, 0 + h, ts(ch, CHUNK)],
                                     start=(h == 0), stop=(h == NH - 1))
            nc.scalar.activation(out=est[:, tt], in_=ps[:], func=AF.Exp,
                                 scale=1.0 / 16.0)

        # ---------------- softmax denominator (replicated) ----------------
        ps_cs = ps_big.tile([P, N], F32, tag="big")
        for tt in range(NT):
            for ch in range(NCH):
                nc.tensor.matmul(ps_cs[:, ts(ch, CHUNK)], s_ones[:],
                                 est[:, tt, ts(ch, CHUNK)],
                                 start=(tt == 0), stop=(tt == NT - 1))
        recip = p_recip.tile([P, N], F32, tag="recip")
        nc.vector.reciprocal(recip[:], ps_cs[:])

        # ---------------- attn @ v ----------------
        outt = p_outt.tile([P, NH, N], BF16, tag="outt")
        for m in range(NH):
            ps = ps_big.tile([P, N], F32, tag="big")
            for tt in range(NT):
                for ch in range(NCH):
                    nc.tensor.matmul(ps[:, ts(ch, CHUNK)],
                                     vt[:, tt, ts(m, P)],
                                     est[:, tt, ts(ch, CHUNK)],
                                     start=(tt == 0), stop=(tt == NT - 1))
            # normalize during copyback
            nc.vector.tensor_tensor(outt[:, m], ps[:], recip[:], OP.mult)

        # ---------------- out projection + residual ----------------
        for m in range(NH):
            ps = ps_big.tile([P, N], F32, tag="big")
            for ch in range(NCH):
                for h in range(NH):
                    nc.tensor.matmul(ps[:, ts(ch, CHUNK)],
                                     s_woT[:, h, ts(m, P)],
                                     outt[:, h, ts(ch, CHUNK)],
                                     start=(h == 0), stop=(h == NH - 1))
            fin = p_fin.tile([P, N], F32, tag="fin")
            nc.vector.tensor_tensor(fin[:], ps[:], xnfb[:, m], OP.add)
            nc.sync.dma_start(out_ap[img, m].rearrange("p n -> p n"), fin[:])


def _build():
    nc = bacc.Bacc("TRN2", debug=False, num_devices=N_CORES)
    t = {}
    t["x"] = nc.dram_tensor("x", [IMGS, NH, P, N], F32, kind="ExternalInput").ap()
    t["wqkT"] = nc.dram_tensor("wqkT", [NH, P, 512], BF16, kind="ExternalInput").ap()
    t["wvT"] = nc.dram_tensor("wvT", [NH, P, C], BF16, kind="ExternalInput").ap()
    t["woT"] = nc.dram_tensor("woT", [NH, P, C], BF16, kind="ExternalInput").ap()
    t["bqk"] = nc.dram_tensor("bqk", [4, P], F32, kind="ExternalInput").ap()
    t["gnw"] = nc.dram_tensor("gnw", [NH, P], F32, kind="ExternalInput").ap()
    t["gnbfb"] = nc.dram_tensor("gnbfb", [NH, P, 2], F32, kind="ExternalInput").ap()
    t["ind"] = nc.dram_tensor("ind", [NH, P, GROUPS], F32, kind="ExternalInput").ap()
    t["indT"] = nc.dram_tensor("indT", [GROUPS, NH, P], F32, kind="ExternalInput").ap()
    t["out"] = nc.dram_tensor("out", [IMGS, NH, P, N], F32, kind="ExternalOutput").ap()
    with tile.TileContext(nc) as tc:
        with ExitStack() as ctx:
            _emit(ctx, tc, t)
    nc.compile()
    return nc


def _host_inputs(x, gn_w, gn_b, qkv_w, qkv_b, out_w, out_b):
    """Build the per-core input maps (host-side weight prep)."""
    x = np.asarray(x, dtype=np.float32).reshape(B, C, N)
    gn_w = np.asarray(gn_w, dtype=np.float32)
    gn_b = np.asarray(gn_b, dtype=np.float32)
    qkv_w = np.asarray(qkv_w, dtype=np.float32)
    qkv_b = np.asarray(qkv_b, dtype=np.float32)
    out_w = np.asarray(out_w, dtype=np.float32)
    out_b = np.asarray(out_b, dtype=np.float32)

    bf = ml_dtypes.bfloat16
    wqkT = np.ascontiguousarray(qkv_w[:512].T).reshape(NH, P, 512).astype(bf)
    wvT = np.ascontiguousarray(qkv_w[512:].T).reshape(NH, P, C).astype(bf)
    woT = np.ascontiguousarray(out_w.T).reshape(NH, P, C).astype(bf)
    bqk = qkv_b[:512].reshape(4, P).astype(np.float32)
    fb = (out_w @ qkv_b[512:] + out_b).astype(np.float32)
    gnbfb = np.stack([gn_b, gn_b + fb], axis=-1).reshape(NH, P, 2).astype(np.float32)
    gnw = gn_w.reshape(NH, P).astype(np.float32)

    ind = np.zeros((NH, P, GROUPS), np.float32)
    indT = np.zeros((GROUPS, NH, P), np.float32)
    cpg = C // GROUPS  # channels per group = 32
    for h in range(NH):
        for p in range(P):
            g = (h * P + p) // cpg
            ind[h, p, g] = 1.0 / cpg
            indT[g, h, p] = 1.0

    shared = dict(wqkT=wqkT, wvT=wvT, woT=woT, bqk=bqk, gnw=gnw,
                  gnbfb=gnbfb, ind=ind, indT=indT)
    in_maps = []
    for core in range(N_CORES):
        xs = x[core * IMGS:(core + 1) * IMGS].reshape(IMGS, NH, P, N)
        in_maps.append(dict(shared, x=np.ascontiguousarray(xs)))
    return in_maps


_NC_CACHE = {}


def _get_nc():
    if "nc" not in _NC_CACHE:
        _NC_CACHE["nc"] = _build()
    return _NC_CACHE["nc"]


def kernel(x, gn_w, gn_b, qkv_w, qkv_b, out_w, out_b, _trace=False, _tmpdir=None):
    nc = _get_nc()
    in_maps = _host_inputs(x, gn_w, gn_b, qkv_w, qkv_b, out_w, out_b)
    res = run_bass_kernel_spmd(nc, in_maps, core_ids=list(range(N_CORES)),
                               trace=_trace, tmpdir=_tmpdir)
    out = np.concatenate([r["out"].reshape(IMGS, C, H, W) for r in res.results])
    if _trace:
        kernel.last_results = res
    return out


# revision 11
# speedup vs baseline: 3386.4405x; 3386.4405x over previous
"""Trainium2 Bass kernel for GroupNorm -> self-attention -> proj + residual.

Reference computation (per image, b=32 total, data-parallel over 8 cores):
    xn    = GroupNorm(x, 8 groups, affine)              [c=256, n=1024]
    qkv   = qkv_w @ xn + qkv_b                          (1x1 conv)
    st    = k^T q / sqrt(c)   (scores transposed: [nk, nq])
    est   = exp(st)           (softmax without max-subtraction; randn data)
    den   = colsum(est)       (ones-matmul, replicated across partitions)
    outT  = (v^T est) / den   [c, nq]
    fin   = out_w @ outT + (out_w @ v_b + out_b) + xn

Layout choices:
  - x/xn/q/k/outT/final live as [c-half on partitions, n free] (2 tiles).
  - v is produced directly transposed ([n on partitions, c free]) by using
    xn as the matmul lhsT, so no on-chip transposes are needed anywhere.
  - softmax denominator: ones[128,128] lhsT matmul accumulated over the 8
    est K-tiles gives the column sum replicated on every partition, which a
    plain tensor_tensor multiply can then consume.
  - 1/den is folded into the outT PSUM->SBUF copyback; the v-bias and
    out-proj bias fold into a host-precomputed per-channel vector fb,
    which is folded into the GroupNorm-residual tile (xn + fb).
Matmul inputs are bf16 (fp32 accumulation in PSUM); GroupNorm statistics,
softmax sums and the residual path stay fp32.
"""

import numpy as np
import ml_dtypes
from contextlib import ExitStack

import concourse.bass as bass
import concourse.tile as tile
import concourse.mybir as mybir
from concourse import bacc
from concourse.bass import ts
from concourse.bass_utils import run_bass_kernel_spmd

P = 128
N_CORES = 8
B, C, H, W = 32, 256, 32, 32
N = H * W                      # 1024 pixels
IMGS = B // N_CORES            # 4 images per core
NH = C // P                    # 2 channel halves
NT = N // P                    # 8 pixel tiles
GROUPS = 8
EPS = 1e-5
F32 = mybir.dt.float32
BF16 = mybir.dt.bfloat16
AF = mybir.ActivationFunctionType
OP = mybir.AluOpType
CHUNK = 512                    # matmul moving free dim (one PSUM bank)
NCH = N // CHUNK               # 2 chunks


def _emit(ctx: ExitStack, tc: tile.TileContext, t: dict, reps: int = 1):
    nc = tc.nc

    singles = ctx.enter_context(tc.tile_pool(name="singles", bufs=1))
    p_x = ctx.enter_context(tc.tile_pool(name="p_x", bufs=2))
    p_stats = ctx.enter_context(tc.tile_pool(name="p_stats", bufs=4))
    p_xnb = ctx.enter_context(tc.tile_pool(name="p_xnb", bufs=2))
    p_xnfb = ctx.enter_context(tc.tile_pool(name="p_xnfb", bufs=2))
    p_qk = ctx.enter_context(tc.tile_pool(name="p_qk", bufs=2))
    p_vt = ctx.enter_context(tc.tile_pool(name="p_vt", bufs=2))
    p_est = ctx.enter_context(tc.tile_pool(name="p_est", bufs=2))
    p_recip = ctx.enter_context(tc.tile_pool(name="p_recip", bufs=2))
    p_outt = ctx.enter_context(tc.tile_pool(name="p_outt", bufs=2))
    p_fin = ctx.enter_context(tc.tile_pool(name="p_fin", bufs=4))
    ps_big = ctx.enter_context(tc.tile_pool(name="ps_big", bufs=3, space="PSUM"))
    ps_sm = ctx.enter_context(tc.tile_pool(name="ps_sm", bufs=2, space="PSUM"))

    # ---- load constants / weights into SBUF once ----
    s_wqkT = singles.tile([P, NH, 512], BF16)
    nc.sync.dma_start(s_wqkT[:], t["wqkT"].rearrange("h p o -> p h o"))
    s_wvT = singles.tile([P, NH, C], BF16)
    nc.sync.dma_start(s_wvT[:], t["wvT"].rearrange("h p o -> p h o"))
    s_woT = singles.tile([P, NH, C], BF16)
    nc.sync.dma_start(s_woT[:], t["woT"].rearrange("h p o -> p h o"))
    s_bqk = singles.tile([P, 4], F32)
    nc.sync.dma_start(s_bqk[:], t["bqk"].rearrange("j p -> p j"))
    s_gnw = singles.tile([P, NH], F32)
    nc.sync.dma_start(s_gnw[:], t["gnw"].rearrange("h p -> p h"))
    s_gnbfb = singles.tile([P, NH, 2], F32)  # col0 = gn_b, col1 = gn_b + fb
    nc.sync.dma_start(s_gnbfb[:], t["gnbfb"].rearrange("h p k -> p h k"))
    s_ind = singles.tile([P, NH, GROUPS], F32)
    nc.sync.dma_start(s_ind[:], t["ind"].rearrange("h p g -> p h g"))
    s_indT = singles.tile([GROUPS, NH, P], F32)
    nc.sync.dma_start(s_indT[:], t["indT"])
    s_ones = singles.tile([P, P], BF16)
    nc.vector.memset(s_ones[:], 1.0)
    s_eps = singles.tile([GROUPS, 1], F32)
    nc.vector.memset(s_eps[:], EPS)

    x_ap = t["x"]       # [IMGS, NH, P, N]
    out_ap = t["out"]   # [IMGS, NH, P, N]

    if reps > 1:
        loop = ctx.enter_context(tc.For_i(0, reps, 1))  # noqa: F841 (timing loop)

    for img in range(IMGS):
        # ---------------- GroupNorm ----------------
        x_t = p_x.tile([P, NH, N], F32, tag="x")
        nc.sync.dma_start(x_t[:], x_ap[img].rearrange("h p n -> p h n"))

        # per-channel mean/var via bn_stats (free dim limited to 512)
        mm = p_stats.tile([P, NH, 2], F32, tag="mm")  # col0 mean, col1 E[x^2]
        for h in range(NH):
            st6 = p_stats.tile([P, 2, 6], F32, tag="st6")
            xv = x_t[:, h].rearrange("p (s f) -> p s f", f=512)
            for s in range(2):
                nc.vector.bn_stats(out=st6[:, s, :], in_=xv[:, s, :])
            mv = p_stats.tile([P, 2], F32, tag="mv")
            nc.vector.bn_aggr(out=mv[:], in_=st6[:])
            nc.vector.tensor_copy(mm[:, h, 0:1], mv[:, 0:1])
            # E[x^2] = var + mean^2
            nc.vector.tensor_tensor(mm[:, h, 1:2], mv[:, 0:1], mv[:, 0:1], OP.mult)
            nc.vector.tensor_tensor(mm[:, h, 1:2], mm[:, h, 1:2], mv[:, 1:2], OP.add)

        # group stats: [8, 2] = sum_h ind[h].T @ mm[h]  (values 1/32 -> mean)
        psg = ps_sm.tile([GROUPS, 2], F32, tag="sm")
        for h in range(NH):
            nc.tensor.matmul(psg[:], s_ind[:, h, :], mm[:, h, :],
                             start=(h == 0), stop=(h == NH - 1))
        grp = p_stats.tile([GROUPS, 2], F32, tag="grp")  # col0 mu, col1 rstd
        nc.vector.tensor_copy(grp[:, 0:1], psg[:, 0:1])
        nc.vector.tensor_copy(grp[:, 1:2], psg[:, 1:2])
        musq = p_stats.tile([GROUPS, 1], F32, tag="musq")
        nc.vector.tensor_tensor(musq[:], grp[:, 0:1], grp[:, 0:1], OP.mult)
        nc.vector.tensor_tensor(grp[:, 1:2], grp[:, 1:2], musq[:], OP.subtract)
        nc.scalar.activation(out=grp[:, 1:2], in_=grp[:, 1:2], func=AF.Sqrt,
                             bias=s_eps[:], scale=1.0)
        nc.vector.reciprocal(grp[:, 1:2], grp[:, 1:2])

        # broadcast group (mu, rstd) back to channels and build affine a,b
        ab = p_stats.tile([P, NH, 3], F32, tag="ab")  # a, b, b+fb
        for h in range(NH):
            psb = ps_sm.tile([P, 2], F32, tag="sm")
            nc.tensor.matmul(psb[:], s_indT[:, h, :], grp[:], start=True, stop=True)
            a = ab[:, h, 0:1]
            nc.vector.tensor_tensor(a, psb[:, 1:2], s_gnw[:, h:h + 1], OP.mult)
            mua = ab[:, h, 1:2]
            nc.vector.tensor_tensor(mua, psb[:, 0:1], a, OP.mult)
            # b = gn_b - mu*a ; b_fb = (gn_b + fb) - mu*a
            nc.vector.tensor_tensor(ab[:, h, 2:3], s_gnbfb[:, h, 1:2], mua, OP.subtract)
            nc.vector.tensor_tensor(mua, s_gnbfb[:, h, 0:1], mua, OP.subtract)

        # apply: xnb = bf16(x*a + b) on ACT; xnfb = f32(x*a + (b+fb)) on DVE
        xnb = p_xnb.tile([P, NH, N], BF16, tag="xnb")
        xnfb = p_xnfb.tile([P, NH, N], F32, tag="xnfb")
        for h in range(NH):
            nc.scalar.activation(out=xnb[:, h], in_=x_t[:, h], func=AF.Identity,
                                 bias=ab[:, h, 1:2], scale=ab[:, h, 0:1])
            nc.vector.tensor_scalar(out=xnfb[:, h], in0=x_t[:, h],
                                    scalar1=ab[:, h, 0:1], scalar2=ab[:, h, 2:3],
                                    op0=OP.mult, op1=OP.add)

        # ---------------- QKV ----------------
        # q,k in [c, n] layout: psum[j] = sum_h wqkT[:,h,128j:].T @ xnb[:,h,:]
        qk = p_qk.tile([P, 4, N], BF16, tag="qk")  # j=0,1 -> q ; j=2,3 -> k
        for j in range(4):
            ps = ps_big.tile([P, N], F32, tag="big")
            for ch in range(NCH):
                for h in range(NH):
                    nc.tensor.matmul(ps[:, ts(ch, CHUNK)],
                                     s_wqkT[:, h, ts(j, P)],
                                     xnb[:, h, ts(ch, CHUNK)],
                                     start=(h == 0), stop=(h == NH - 1))
            nc.any.tensor_scalar(out=qk[:, j], in0=ps[:],
                                 scalar1=s_bqk[:, j:j + 1], scalar2=None,
                                 op0=OP.add)

        # vT in [n, c] layout: psum[t] = sum_h xnb[:,h,128t:].T @ wvT[:,h,:]
        vt = p_vt.tile([P, NT, C], BF16, tag="vt")
        for tt in range(NT):
            ps = ps_sm.tile([P, C], F32, tag="sm")
            for h in range(NH):
                nc.tensor.matmul(ps[:], xnb[:, h, ts(tt, P)], s_wvT[:, h, :],
                                 start=(h == 0), stop=(h == NH - 1))
            nc.any.tensor_copy(out=vt[:, tt], in_=ps[:])

        # ---------------- scores^T + exp ----------------
        est = p_est.tile([P, NT, N], BF16, tag="est")
        for tt in range(NT):
            ps = ps_big.tile([P, N], F32, tag="big")
            for ch in range(NCH):
                for h in range(NH):
                    nc.tensor.matmul(ps[:, ts(ch, CHUNK)],
                                     qk[:, 2 + h, ts(tt, P)],
                                     qk[:, 0 + h, ts(ch, CHUNK)],
                                     start=(h == 0), stop=(h == NH - 1))
            nc.scalar.activation(out=est[:, tt], in_=ps[:], func=AF.Exp,
                                 scale=1.0 / 16.0)

        # ---------------- softmax denominator (replicated) ----------------
        ps_cs = ps_big.tile([P, N], F32, tag="big")
        for tt in range(NT):
            for ch in range(NCH):
                nc.tensor.matmul(ps_cs[:, ts(ch, CHUNK)], s_ones[:],
                                 est[:, tt, ts(ch, CHUNK)],
                                 start=(tt == 0), stop=(tt == NT - 1))
        recip = p_recip.tile([P, N], F32, tag="recip")
        nc.vector.reciprocal(recip[:], ps_cs[:])

        # ---------------- attn @ v ----------------
        outt = p_outt.tile([P, NH, N], BF16, tag="outt")
        for m in range(NH):
            ps = ps_big.tile([P, N], F32, tag="big")
            for tt in range(NT):
                for ch in range(NCH):
                    nc.tensor.matmul(ps[:, ts(ch, CHUNK)],
                                     vt[:, tt, ts(m, P)],
                                     est[:, tt, ts(ch, CHUNK)],
                                     start=(tt == 0), stop=(tt == NT - 1))
            # normalize during copyback
            nc.vector.tensor_tensor(outt[:, m], ps[:], recip[:], OP.mult)

        # ---------------- out projection + residual ----------------
        for m in range(NH):
            ps = ps_big.tile([P, N], F32, tag="big")
            for ch in range(NCH):
                for h in range(NH):
                    nc.tensor.matmul(ps[:, ts(ch, CHUNK)],
                                     s_woT[:, h, ts(m, P)],
                                     outt[:, h, ts(ch, CHUNK)],
                                     start=(h == 0), stop=(h == NH - 1))
            fin = p_fin.tile([P, N], F32, tag="fin")
            nc.vector.tensor_tensor(fin[:], ps[:], xnfb[:, m], OP.add)
            nc.sync.dma_start(out_ap[img, m].rearrange("p n -> p n"), fin[:])


def _build(reps: int = 1):
    nc = bacc.Bacc("TRN2", debug=False, num_devices=N_CORES)
    t = {}
    t["x"] = nc.dram_tensor("x", [IMGS, NH, P, N], F32, kind="ExternalInput").ap()
    t["wqkT"] = nc.dram_tensor("wqkT", [NH, P, 512], BF16, kind="ExternalInput").ap()
    t["wvT"] = nc.dram_tensor("wvT", [NH, P, C], BF16, kind="ExternalInput").ap()
    t["woT"] = nc.dram_tensor("woT", [NH, P, C], BF16, kind="ExternalInput").ap()
    t["bqk"] = nc.dram_tensor("bqk", [4, P], F32, kind="ExternalInput").ap()
    t["gnw"] = nc.dram_tensor("gnw", [NH, P], F32, kind="ExternalInput").ap()
    t["gnbfb"] = nc.dram_tensor("gnbfb", [NH, P, 2], F32, kind="ExternalInput").ap()
    t["ind"] = nc.dram_tensor("ind", [NH, P, GROUPS], F32, kind="ExternalInput").ap()
    t["indT"] = nc.dram_tensor("indT", [GROUPS, NH, P], F32, kind="ExternalInput").ap()
    t["out"] = nc.dram_tensor("out", [IMGS, NH, P, N], F32, kind="ExternalOutput").ap()
    with tile.TileContext(nc) as tc:
        with ExitStack() as ctx:
            _emit(ctx, tc, t, reps=reps)
    nc.compile()
    return nc


def _host_inputs(x, gn_w, gn_b, qkv_w, qkv_b, out_w, out_b):
    """Build the per-core input maps (host-side weight prep)."""
    x = np.asarray(x, dtype=np.float32).reshape(B, C, N)
    gn_w = np.asarray(gn_w, dtype=np.float32)
    gn_b = np.asarray(gn_b, dtype=np.float32)
    qkv_w = np.asarray(qkv_w, dtype=np.float32)
    qkv_b = np.asarray(qkv_b, dtype=np.float32)
    out_w = np.asarray(out_w, dtype=np.float32)
    out_b = np.asarray(out_b, dtype=np.float32)

    bf = ml_dtypes.bfloat16
    wqkT = np.ascontiguousarray(qkv_w[:512].T).reshape(NH, P, 512).astype(bf)
    wvT = np.ascontiguousarray(qkv_w[512:].T).reshape(NH, P, C).astype(bf)
    woT = np.ascontiguousarray(out_w.T).reshape(NH, P, C).astype(bf)
    bqk = qkv_b[:512].reshape(4, P).astype(np.float32)
    fb = (out_w @ qkv_b[512:] + out_b).astype(np.float32)
    gnbfb = np.stack([gn_b, gn_b + fb], axis=-1).reshape(NH, P, 2).astype(np.float32)
    gnw = gn_w.reshape(NH, P).astype(np.float32)

    ind = np.zeros((NH, P, GROUPS), np.float32)
    indT = np.zeros((GROUPS, NH, P), np.float32)
    cpg = C // GROUPS  # channels per group = 32
    for h in range(NH):
        for p in range(P):
            g = (h * P + p) // cpg
            ind[h, p, g] = 1.0 / cpg
            indT[g, h, p] = 1.0

    shared = dict(wqkT=wqkT, wvT=wvT, woT=woT, bqk=bqk, gnw=gnw,
                  gnbfb=gnbfb, ind=ind, indT=indT)
    in_maps = []
    for core in range(N_CORES):
        xs = x[core * IMGS:(core + 1) * IMGS].reshape(IMGS, NH, P, N)
        in_maps.append(dict(shared, x=np.ascontiguousarray(xs)))
    return in_maps


_NC_CACHE = {}


def _get_nc():
    if "nc" not in _NC_CACHE:
        _NC_CACHE["nc"] = _build()
    return _NC_CACHE["nc"]


def kernel(x, gn_w, gn_b, qkv_w, qkv_b, out_w, out_b, _reps=1):
    if _reps > 1:
        nc = _build(reps=_reps)
    else:
        nc = _get_nc()
    in_maps = _host_inputs(x, gn_w, gn_b, qkv_w, qkv_b, out_w, out_b)
    res = run_bass_kernel_spmd(nc, in_maps, core_ids=list(range(N_CORES)))
    out = np.concatenate([r["out"].reshape(IMGS, C, H, W) for r in res.results])
    kernel.last_results = res
    return out
